# revision 28
# baseline (speedup 1.0000x reference)
"""AdaptivePoolAttention Trainium2 kernel (8 NeuronCores, SPMD).

Key algebraic restructure: AdaptiveAvgPool3d over spatial (H,W) commutes with
the qkv linear projection, so we pool x first (mean over H*W) and run the qkv
GEMM on the pooled (B,T,D) tensor. This turns the dominant-GEMM problem into a
memory-bound spatial reduction plus a small attention tail.

Sharding: core c handles batch b=c//2, token half c%2 (32 of 64 tokens).
 - Phase 1: each core pools its 32 tokens (spatial mean via a bf16 selector
   matmul on the TensorEngine, accumulated in PSUM).
 - Pairwise AllGather (cores 2c, 2c+1) of the pooled halves -> full (64, 768).
 - Phase 2b (overlaps the collective): q projection + LN for the local 32
   query tokens, plus the temporal rel-pos bias. The bias
   bias[t,s,h] = sum_d q[t,h,d] * rel_pos_t[t-s+63, d] is computed as one
   GEMM G = q @ R^T over the 127 distinct distances, then gathered into
   (t, h, s) layout with a single affine-strided DMA through DRAM
   (address = t*1535 + h*128 + s + 63 after a per-core column flip of R).
 - Phase 3: k/v projection + LN for all 64 tokens, per-head attention,
   softmax (no max-subtraction: |logits| < 5), A@V, residual, out projection.
Host side shards/preps inputs (incl. fp32->bf16 for the big operands) and
concatenates the 8 disjoint (32, 768) output row-blocks.
"""

import numpy as np
from contextlib import ExitStack

B, T, NH, HD, D = 4, 64, 12, 64, 768
S = 196            # 14*14 spatial positions
TLOC = 32          # tokens per core
NROWS = TLOC * S   # 6272 rows of x per core
NT = 49            # 128-row tiles per core
G = 7              # tiles per DMA group
KB = D // 128      # 6 contraction tiles of 128
NDIST = 2 * T - 1  # 127 distinct temporal distances
SCALE = HD ** -0.5
LN_EPS = 1e-5
N_CORES = 8

_BUILD_CACHE = {}


def _build_nc():
    import concourse.bass as bass
    import concourse.bacc as bacc
    import concourse.tile as tile
    import concourse.mybir as mybir
    from concourse.masks import make_identity
    from concourse.tile_rust import add_dep_helper

    f32 = mybir.dt.float32
    bf16 = mybir.dt.bfloat16

    nc = bacc.Bacc(
        "TRN2", target_bir_lowering=False, debug=False, num_devices=N_CORES,
    )

    xloc = nc.declare_dram_parameter("xloc", [NROWS, D], bf16, isOutput=False)
    sel = nc.declare_dram_parameter("sel", [128, NT, TLOC], bf16, isOutput=False)
    wq = nc.declare_dram_parameter("wq", [D, D], bf16, isOutput=False)
    wkv = nc.declare_dram_parameter("wkv", [D, 2 * D], bf16, isOutput=False)
    wp = nc.declare_dram_parameter("wp", [D, D], bf16, isOutput=False)
    rptt = nc.declare_dram_parameter("rptt", [HD, 128], bf16, isOutput=False)
    gq = nc.declare_dram_parameter("gq", [TLOC, D], f32, isOutput=False)
    bq = nc.declare_dram_parameter("bq", [TLOC, D], f32, isOutput=False)
    gkcol = nc.declare_dram_parameter("gkcol", [HD, 1], f32, isOutput=False)
    bkrow = nc.declare_dram_parameter("bkrow", [TLOC, D], f32, isOutput=False)
    gvrow = nc.declare_dram_parameter("gvrow", [TLOC, D], f32, isOutput=False)
    bvrow = nc.declare_dram_parameter("bvrow", [TLOC, D], f32, isOutput=False)
    bproj = nc.declare_dram_parameter("bproj", [TLOC, D], f32, isOutput=False)
    out_ext = nc.declare_dram_parameter("out", [TLOC, D], f32, isOutput=True)

    with ExitStack() as ctx:
        tc = ctx.enter_context(tile.TileContext(nc))
        const = ctx.enter_context(tc.tile_pool(name="const", bufs=1))
        xp = ctx.enter_context(tc.tile_pool(name="xp", bufs=2))
        sb = ctx.enter_context(tc.tile_pool(name="sb", bufs=1))
        pg = ctx.enter_context(tc.tile_pool(name="pg", bufs=2, space="PSUM"))
        pt = ctx.enter_context(tc.tile_pool(name="pt", bufs=2, space="PSUM"))
        dram = ctx.enter_context(tc.tile_pool(name="dram", bufs=1, space="DRAM"))

        ident = const.tile([128, 128], bf16, tag="ident")
        make_identity(nc, ident)
        eps_sb = const.tile([128, 1], f32, tag="eps")
        nc.vector.memset(eps_sb, LN_EPS)
        zero_sb = const.tile([128, 1], f32, tag="zero")
        nc.vector.memset(zero_sb, 0.0)

        # ---- phase 1 inputs first on the sync ring: sel, then x groups ----
        sel_sb = const.tile([128, NT, TLOC], bf16, tag="sel")
        nc.sync.dma_start(out=sel_sb, in_=sel.ap())

        GROUPS = (1, 2, 4, 7, 7, 7, 7, 7, 7)  # progressive sizes, sum = NT
        x_t = xloc.ap().rearrange("(n p) d -> n p d", p=128)
        m_psum = pg.tile([TLOC, D], f32, tag="g")
        x_dmas = []
        t_base = 0
        for g, gsz in enumerate(GROUPS):
            xt = xp.tile([128, G, D], bf16, tag="x")
            ring = nc.sync if g % 2 == 0 else nc.scalar
            src_ap = bass.AP(
                tensor=x_t.tensor,
                offset=x_t.offset + t_base * 128 * D,
                ap=[[D, 128], [128 * D, gsz], [1, D]],
            )
            x_dmas.append(ring.dma_start(out=xt[:, 0:gsz, :], in_=src_ap))
            for i in range(gsz):
                ti = t_base + i
                for c0, cw in ((0, 512), (512, 256)):
                    nc.tensor.matmul(
                        m_psum[:, c0:c0 + cw],
                        sel_sb[:, ti, :],
                        xt[:, i, c0:c0 + cw],
                        start=(ti == 0),
                        stop=(ti == NT - 1),
                    )
            t_base += gsz
        m_sb = sb.tile([TLOC, D], bf16, tag="m")
        nc.vector.tensor_copy(out=m_sb, in_=m_psum)

        # ---- weights and constants strictly after BOTH rings' x streams ----
        # (anything streaming during phase 1 steals HBM bandwidth from x: the
        # pair of cores shares one HBM stack, so the x tail otherwise stalls
        # the last pooling matmuls by ~10us)
        last_sync = x_dmas[-1] if (len(GROUPS) - 1) % 2 == 0 else x_dmas[-2]
        last_scal = x_dmas[-1] if (len(GROUPS) - 1) % 2 == 1 else x_dmas[-2]

        def after_x(wd):
            add_dep_helper(wd.ins, last_sync.ins, sync=False, reason="x before w")
            add_dep_helper(wd.ins, last_scal.ins, sync=False, reason="x before w")

        # issue in need-order: q path first, kv/proj later
        wq_sb = const.tile([128, KB, D], bf16, tag="wq")
        after_x(nc.sync.dma_start(out=wq_sb, in_=wq.ap().rearrange("(k p) e -> p k e", p=128)))
        rptt_sb = const.tile([HD, 128], bf16, tag="rptt")
        after_x(nc.scalar.dma_start(out=rptt_sb, in_=rptt.ap()))
        gq_sb = const.tile([TLOC, D], f32, tag="gq")
        after_x(nc.scalar.dma_start(out=gq_sb, in_=gq.ap()))
        bq_sb = const.tile([TLOC, D], f32, tag="bq")
        after_x(nc.scalar.dma_start(out=bq_sb, in_=bq.ap()))
        bkrow_sb = const.tile([TLOC, D], f32, tag="bkrow")
        after_x(nc.scalar.dma_start(out=bkrow_sb, in_=bkrow.ap()))
        bvrow_sb = const.tile([TLOC, D], f32, tag="bvrow")
        after_x(nc.scalar.dma_start(out=bvrow_sb, in_=bvrow.ap()))
        wkv_sb = const.tile([128, KB, 2 * D], bf16, tag="wkv")
        after_x(nc.scalar.dma_start(out=wkv_sb, in_=wkv.ap().rearrange("(k p) e -> p k e", p=128)))
        wp_sb = const.tile([128, KB, D], bf16, tag="wp")
        after_x(nc.sync.dma_start(out=wp_sb, in_=wp.ap().rearrange("(k p) e -> p k e", p=128)))
        gkcol_sb = const.tile([HD, 1], f32, tag="gkcol")
        after_x(nc.sync.dma_start(out=gkcol_sb, in_=gkcol.ap()))
        gvrow_sb = const.tile([TLOC, D], f32, tag="gvrow")
        after_x(nc.sync.dma_start(out=gvrow_sb, in_=gvrow.ap()))
        bproj_sb = const.tile([TLOC, D], f32, tag="bproj")
        after_x(nc.sync.dma_start(out=bproj_sb, in_=bproj.ap()))

        # ---- tiny dummy AllGather: wakes ncfw so the real collective
        # triggers with ~1us delay instead of ~11us ----
        agw_in = dram.tile([1, 32], bf16, tag="agwi")
        agw_out = dram.tile([2, 32], bf16, tag="agwo")
        nc.gpsimd.dma_start(out=agw_in, in_=ident[0:1, 0:32])
        nc.gpsimd.collective_compute(
            "AllGather",
            mybir.AluOpType.bypass,
            replica_groups=[[0, 1], [2, 3], [4, 5], [6, 7]],
            ins=[agw_in.opt()],
            outs=[agw_out.opt()],
        )

        # ---- pairwise AllGather of pooled halves (bf16) ----
        ag_in = dram.tile([TLOC, D], bf16, tag="agi")
        ag_out = dram.tile([T, D], bf16, tag="ago")
        nc.gpsimd.dma_start(out=ag_in, in_=m_sb)
        nc.gpsimd.collective_compute(
            "AllGather",
            mybir.AluOpType.bypass,
            replica_groups=[[0, 1], [2, 3], [4, 5], [6, 7]],
            ins=[ag_in.opt()],
            outs=[ag_out.opt()],
        )
        mf_sb = sb.tile([T, D], bf16, tag="mf")
        nc.sync.dma_start(out=mf_sb, in_=ag_out)

        def bcast_free(ap2d, inner):
            # (P, F) AP -> (P, F, inner) AP with stride-0 innermost broadcast
            return bass.AP(
                tensor=ap2d.tensor,
                offset=ap2d.offset,
                ap=[*ap2d.ap, [0, inner]],
            )

        def layer_norm(src_psum, n_part, n_groups, g_tile, b_tile, out_tile, nm):
            # src (n_part, n_groups*64): per-64-group LN, batched DVE ops
            src3 = src_psum.rearrange("p (g d) -> p g d", g=n_groups)
            mean = sb.tile([n_part, n_groups], f32, tag=f"{nm}_mean")
            nc.vector.reduce_sum(out=mean, in_=src3, axis=mybir.AxisListType.X)
            nc.vector.tensor_scalar_mul(out=mean, in0=mean, scalar1=1.0 / HD)
            xc = sb.tile([n_part, n_groups, HD], f32, tag=f"{nm}_xc")
            nc.vector.tensor_tensor(
                out=xc, in0=src3, in1=bcast_free(mean[:], HD),
                op=mybir.AluOpType.subtract,
            )
            sq = sb.tile([n_part, n_groups, HD], f32, tag=f"{nm}_sq")
            nc.vector.tensor_mul(out=sq, in0=xc, in1=xc)
            var = sb.tile([n_part, n_groups], f32, tag=f"{nm}_var")
            nc.vector.reduce_sum(out=var, in_=sq, axis=mybir.AxisListType.X)
            # std = sqrt(var/HD + eps); rstd = 1/std
            nc.scalar.activation(
                out=var, in_=var, func=mybir.ActivationFunctionType.Sqrt,
                bias=eps_sb[:n_part], scale=1.0 / HD,
            )
            nc.vector.reciprocal(out=var, in_=var)
            if g_tile is None:
                nc.vector.tensor_tensor(
                    out=out_tile.rearrange("p (g d) -> p g d", g=n_groups),
                    in0=xc, in1=bcast_free(var[:], HD),
                    op=mybir.AluOpType.mult,
                )
                return
            nc.vector.tensor_tensor(
                out=xc, in0=xc, in1=bcast_free(var[:], HD),
                op=mybir.AluOpType.mult,
            )
            xcf = xc.rearrange("p g d -> p (g d)")
            nc.vector.tensor_mul(out=xcf, in0=xcf, in1=g_tile)
            nc.vector.tensor_add(out=out_tile, in0=xcf, in1=b_tile)

        # ---- phase 2b: q path (local tokens; overlaps the collective) ----
        mT_psum = pt.tile([128, KB, TLOC], bf16, tag="t")
        for k in range(KB):
            nc.tensor.matmul(
                mT_psum[:, k, :], m_sb[:, k * 128:(k + 1) * 128],
                ident[:TLOC, :TLOC], is_transpose=True,
            )
        mT_sb = sb.tile([128, KB, TLOC], bf16, tag="mT")
        nc.any.tensor_copy(out=mT_sb, in_=mT_psum)

        q_psum = pg.tile([TLOC, D], f32, tag="g")
        for k in range(KB):
            for c0, cw in ((0, 512), (512, 256)):
                nc.tensor.matmul(
                    q_psum[:, c0:c0 + cw],
                    mT_sb[:, k, :],
                    wq_sb[:, k, c0:c0 + cw],
                    start=(k == 0), stop=(k == KB - 1),
                )
        ln_q = sb.tile([TLOC, D], bf16, tag="lnq")
        layer_norm(q_psum, TLOC, NH, gq_sb, bq_sb, ln_q, "q")

        # q^T in per-head layout (64 d, NH heads, 32 t)
        qbT_psum = pt.tile([HD, NH, TLOC], bf16, tag="t")
        for h in range(NH):
            nc.tensor.matmul(
                qbT_psum[:, h, :], ln_q[:, h * HD:(h + 1) * HD],
                ident[:TLOC, :TLOC], is_transpose=True,
            )
        qbT_sb = sb.tile([HD, NH, TLOC], bf16, tag="qbT")
        nc.any.tensor_copy(out=qbT_sb, in_=qbT_psum)

        # Bq[t,h] = sum_d ln_q[t,h,d] * b_k[d]  (k-LN beta folded into bias)
        qbk = sb.tile([TLOC, NH, HD], f32, tag="qbk")
        nc.vector.tensor_mul(
            out=qbk, in0=ln_q.rearrange("p (g d) -> p g d", g=NH),
            in1=bkrow_sb.rearrange("p (g d) -> p g d", g=NH),
        )
        bq_fold = sb.tile([TLOC, NH], f32, tag="bqf")
        nc.vector.reduce_sum(out=bq_fold, in_=qbk, axis=mybir.AxisListType.X)
        # residual+beta_v tile: ln_q + b_v
        lnq_bv = sb.tile([TLOC, D], bf16, tag="lnqbv")
        nc.vector.tensor_add(out=lnq_bv, in0=ln_q, in1=bvrow_sb)

        # preload the ACT Exp table so the tail's softmax doesn't stall on it
        expwarm = sb.tile([1, 1], f32, tag="expwarm")
        nc.scalar.activation(
            out=expwarm, in_=zero_sb[0:1, :],
            func=mybir.ActivationFunctionType.Exp,
            bias=zero_sb[0:1, :], scale=1.0,
        )

        # rel-pos bias: G[t, h, j] = sum_d q[t,h,d] * Rflip[d, j], then an
        # affine gather through DRAM turns G into bias[t, h, s] (j = 63-t+s).
        g_psum = pg.tile([TLOC, NH, 128], f32, tag="g")
        for h in range(NH):
            nc.tensor.matmul(
                g_psum[:, h, :], qbT_sb[:, h, :], rptt_sb,
                start=True, stop=True,
            )
        g_sb = sb.tile([TLOC, NH, 128], f32, tag="gsb")
        nc.any.tensor_copy(out=g_sb, in_=g_psum)
        g_dram = dram.tile([TLOC, NH, 128], f32, tag="gd")
        nc.sync.dma_start(out=g_dram, in_=g_sb)
        bias_sb = sb.tile([TLOC, NH, T], f32, tag="bias")
        gather_ap = bass.AP(
            tensor=g_dram.tensor,
            offset=g_dram.offset + 63,
            ap=[[NH * 128 - 1, TLOC], [128, NH], [1, T]],
        )
        nc.sync.dma_start(out=bias_sb, in_=gather_ap)
        nc.vector.tensor_tensor(
            out=bias_sb, in0=bias_sb, in1=bcast_free(bq_fold[:], T),
            op=mybir.AluOpType.add,
        )

        # ---- PE warm-hold: keep HAM at K=8/8 through the collective idle
        # window so the attention tail doesn't run at half clock ----
        warm_psum = pg.tile([TLOC, 512], f32, tag="g")
        for _ in range(24):
            nc.tensor.matmul(
                warm_psum, sel_sb[:, 0, :], wp_sb[:, 0, 0:512],
                start=True, stop=True,
            )

        # ---- phase 3: kv path on gathered tokens ----
        mfT_psum = pt.tile([128, KB, T], bf16, tag="t")
        for k in range(KB):
            nc.tensor.matmul(
                mfT_psum[:, k, :], mf_sb[:, k * 128:(k + 1) * 128],
                ident[:T, :T], is_transpose=True,
            )
        mfT_sb = sb.tile([128, KB, T], bf16, tag="mfT")
        nc.vector.tensor_copy(out=mfT_sb, in_=mfT_psum)

        kv_psum = pg.tile([128, D], f32, tag="g")
        for k in range(KB):
            for c0, cw in ((0, 512), (512, 256)):
                nc.tensor.matmul(
                    kv_psum[0:T, c0:c0 + cw],
                    mfT_sb[:, k, :],
                    wkv_sb[:, k, c0:c0 + cw],
                    start=(k == 0), stop=(k == KB - 1),
                )
            for c0, cw in ((0, 512), (512, 256)):
                nc.tensor.matmul(
                    kv_psum[T:128, c0:c0 + cw],
                    mfT_sb[:, k, :],
                    wkv_sb[:, k, D + c0:D + c0 + cw],
                    start=(k == 0), stop=(k == KB - 1),
                )
        # normalized-only LN for k and v together (128 partitions, no gamma/beta)
        ln_kv = sb.tile([128, D], bf16, tag="lnkv")
        layer_norm(kv_psum, 128, NH, None, None, ln_kv, "kv")
        # v slice back to partition base 0 for the A@V matmul rhs
        ln_v = sb.tile([T, D], bf16, tag="lnv")
        nc.vector.tensor_copy(out=ln_v, in_=ln_kv[T:128, :])

        # k^T per head: (64 d, NH, 64 s); gamma_k applied per-partition on copy
        kT_psum = pt.tile([HD, NH, T], bf16, tag="t")
        for h in range(NH):
            nc.tensor.matmul(
                kT_psum[:, h, :], ln_kv[0:T, h * HD:(h + 1) * HD],
                ident[:T, :T], is_transpose=True,
            )
        kT_sb = sb.tile([HD, NH, T], bf16, tag="kT")
        nc.vector.tensor_scalar_mul(
            out=kT_sb.rearrange("p h s -> p (h s)"),
            in0=kT_psum.rearrange("p h s -> p (h s)"),
            scalar1=gkcol_sb,
        )

        # scores = q @ k^T, then add bias, then exp (scaled)
        s_psum = pg.tile([TLOC, NH, T], f32, tag="g")
        for h in range(NH):
            nc.tensor.matmul(
                s_psum[:, h, :], qbT_sb[:, h, :], kT_sb[:, h, :],
                start=True, stop=True,
            )
        s_sb = sb.tile([TLOC, NH, T], f32, tag="ssb")
        nc.vector.tensor_add(out=s_sb, in0=s_psum, in1=bias_sb)
        p_sb = sb.tile([TLOC, NH, T], bf16, tag="p")
        nc.scalar.activation(
            out=p_sb.rearrange("p h s -> p (h s)"),
            in_=s_sb.rearrange("p h s -> p (h s)"),
            func=mybir.ActivationFunctionType.Exp,
            bias=zero_sb[:TLOC], scale=SCALE,
        )
        rsum = sb.tile([TLOC, NH], f32, tag="rsum")
        nc.vector.reduce_sum(out=rsum, in_=p_sb, axis=mybir.AxisListType.X)
        nc.vector.reciprocal(out=rsum, in_=rsum)

        # P^T per head
        pT_psum = pt.tile([T, NH, TLOC], bf16, tag="t")
        for h in range(NH):
            nc.tensor.matmul(
                pT_psum[:, h, :], p_sb[:, h, :],
                ident[:TLOC, :TLOC], is_transpose=True,
            )
        pT_sb = sb.tile([T, NH, TLOC], bf16, tag="pT")
        nc.vector.tensor_copy(out=pT_sb, in_=pT_psum)

        # A@V per head
        o_psum = pg.tile([TLOC, NH, HD], f32, tag="g")
        for h in range(NH):
            nc.tensor.matmul(
                o_psum[:, h, :], pT_sb[:, h, :],
                ln_v[:, h * HD:(h + 1) * HD],
                start=True, stop=True,
            )
        # o = (P@v_hat) * (1/sum) * gamma_v + (ln_q + beta_v), batched wide:
        rg = sb.tile([TLOC, NH, HD], f32, tag="rg")
        nc.vector.tensor_tensor(
            out=rg, in0=gvrow_sb.rearrange("p (g d) -> p g d", g=NH),
            in1=bcast_free(rsum[:], HD), op=mybir.AluOpType.mult,
        )
        o_nrm = sb.tile([TLOC, NH, HD], bf16, tag="onrm")
        nc.vector.tensor_tensor(
            out=o_nrm, in0=o_psum, in1=rg, op=mybir.AluOpType.mult,
        )
        o_sb = sb.tile([TLOC, D], bf16, tag="o")
        nc.vector.tensor_add(
            out=o_sb, in0=o_nrm.rearrange("p h d -> p (h d)"), in1=lnq_bv,
        )

        # o^T then output projection
        oT_psum = pt.tile([128, KB, TLOC], bf16, tag="t")
        for k in range(KB):
            nc.tensor.matmul(
                oT_psum[:, k, :], o_sb[:, k * 128:(k + 1) * 128],
                ident[:TLOC, :TLOC], is_transpose=True,
            )
        oT_sb = sb.tile([128, KB, TLOC], bf16, tag="oT")
        nc.vector.tensor_copy(out=oT_sb, in_=oT_psum)

        proj_psum = pg.tile([TLOC, D], f32, tag="g")
        out_sb = sb.tile([TLOC, D], f32, tag="outsb")
        for c0, cw in ((0, 512), (512, 256)):
            for k in range(KB):
                nc.tensor.matmul(
                    proj_psum[:, c0:c0 + cw],
                    oT_sb[:, k, :],
                    wp_sb[:, k, c0:c0 + cw],
                    start=(k == 0), stop=(k == KB - 1),
                )
            nc.vector.tensor_add(
                out=out_sb[:, c0:c0 + cw], in0=proj_psum[:, c0:c0 + cw],
                in1=bproj_sb[:, c0:c0 + cw],
            )
            nc.sync.dma_start(
                out=out_ext.ap()[:, c0:c0 + cw], in_=out_sb[:, c0:c0 + cw],
            )

    nc.compile()
    return nc


def _host_prep(x, W_qkv, g_q, b_q, g_k, b_k, g_v, b_v, W_proj, b_proj, rel_pos_t):
    import ml_dtypes
    bf = ml_dtypes.bfloat16
    x = np.asarray(x, np.float32)
    W_qkv = np.asarray(W_qkv, np.float32)
    W_proj = np.asarray(W_proj, np.float32)
    rel_pos_t = np.asarray(rel_pos_t, np.float32)

    sel = np.zeros((NROWS, TLOC), np.float32)
    sel[np.arange(NROWS), np.arange(NROWS) // S] = 1.0 / S
    # pre-layout to the SBUF tile shape (128 partitions, NT, TLOC)
    sel = np.ascontiguousarray(
        sel.reshape(NT, 128, TLOC).transpose(1, 0, 2).astype(bf))
    rel_eff = rel_pos_t / SCALE                            # (127, HD)
    wq_b = np.ascontiguousarray(W_qkv[:, :D].astype(bf))
    wkv_b = np.ascontiguousarray(W_qkv[:, D:].astype(bf))
    wp_b = np.ascontiguousarray(W_proj.astype(bf))
    gq_b = np.ascontiguousarray(np.broadcast_to(np.tile(np.asarray(g_q, np.float32), NH), (TLOC, D)))
    bq_b = np.ascontiguousarray(np.broadcast_to(np.tile(np.asarray(b_q, np.float32), NH), (TLOC, D)))
    gk_col = np.ascontiguousarray(np.asarray(g_k, np.float32).reshape(HD, 1))
    bk_b = np.ascontiguousarray(np.broadcast_to(np.tile(np.asarray(b_k, np.float32), NH), (TLOC, D)))
    gv_b = np.ascontiguousarray(np.broadcast_to(np.tile(np.asarray(g_v, np.float32), NH), (TLOC, D)))
    bv_b = np.ascontiguousarray(np.broadcast_to(np.tile(np.asarray(b_v, np.float32), NH), (TLOC, D)))
    bproj_b = np.ascontiguousarray(np.broadcast_to(np.asarray(b_proj, np.float32), (TLOC, D)))

    in_maps = []
    jj = np.arange(128)
    for c in range(N_CORES):
        b = c // 2
        t0 = (c % 2) * TLOC
        # R flipped per core: R_c[d, j] = rel_eff[clip(t0 + 126 - j), d]
        idx = np.clip(t0 + 126 - jj, 0, NDIST - 1)
        rptt_c = np.ascontiguousarray(rel_eff[idx].T.astype(bf))   # (HD, 128)
        in_maps.append({
            "xloc": np.ascontiguousarray(
                x[b, t0:t0 + TLOC].reshape(NROWS, D).astype(bf)),
            "sel": sel,
            "wq": wq_b,
            "wkv": wkv_b,
            "wp": wp_b,
            "rptt": rptt_c,
            "gq": gq_b, "bq": bq_b,
            "gkcol": gk_col, "bkrow": bk_b,
            "gvrow": gv_b, "bvrow": bv_b,
            "bproj": bproj_b,
        })
    return in_maps


def _get_nc():
    if "nc" not in _BUILD_CACHE:
        _BUILD_CACHE["nc"] = _build_nc()
    return _BUILD_CACHE["nc"]


def run_on_device(in_maps, **kw):
    from concourse.bass_utils import run_bass_kernel_spmd
    nc = _get_nc()
    return run_bass_kernel_spmd(nc, in_maps, list(range(N_CORES)), **kw)


def kernel(**inputs):
    in_maps = _host_prep(**inputs)
    res = run_on_device(in_maps)
    out = np.zeros((B, T, D), np.float32)
    for c in range(N_CORES):
        b = c // 2
        t0 = (c % 2) * TLOC
        out[b, t0:t0 + TLOC] = res.results[c]["out"]
    return out
